# revision 30
# baseline (speedup 1.0000x reference)
"""HarmonicCausalSelfAttention on 8 TRN2 NeuronCores.

Sharding: core c -> (batch b = c//2, head-group g = c%2); each core computes
attention for 8 heads of one batch and a full-width partial of the output
projection; the host sums the two partials per batch (the rank-128 c-proj
intermediate is linear, so out = (r_g0 + r_g1) @ cB^T = part_g0 + part_g1).

All-bf16 pipeline (fp8 anywhere fails the 2e-2 gate: logits reach 10 sigma).

  warmup:  ~70 tiny matmuls during the input-DMA wait flip the PE HAM clock
           gate to 2.4 GHz before phase A; small filler matmuls threaded
           through attention keep it warm (throttle_active 160us -> 55us).
  stage1:  t^T[rank, T] = A @ x^T   (x^T per-cc tiles so compute starts
           after the first DMA chunk lands)
  stage2:  Q^T/K^T stored as head-PAIR tiles [128, T] (head 2p on partitions
           0:64, head 2p+1 on 64:128) so the K=64 attention matmuls of the
           two heads run CONCURRENTLY in PE row-groups 0-63/64-127.
  attn:    j-major sweep (one 512-query output bank at a time), strips
           S^T[keys, q] for both heads into one psum [128,2,512] chunk.
           Softmax numerator split across engines:
             - diagonal + early strips: ScalarE exact exp -> bf16, then
               causal 0/1 mask-multiply on DVE (cols 0:128, pair-wide via
               free-dim broadcast)
             - off-diagonal strips kb>=4: DVE "Schraudolph" exp writing the
               bf16 bit pattern: int16 = S*23.0831 + 16256.5 (= exp(S/8) to
               ~3%; softmax renormalization cancels the shared bias)
           PV: psum[65, 512q] += [V_kb | 1]^T @ P^T_kb  (row 64 = denom)
           normalize: native copy of the denom row (custom DVE ops ignore
           partition offsets!) -> reciprocal_approx_fast (DVE) ->
           partition_broadcast (GpSimd) -> tensor_mul (DVE)
  c_proj:  r^T[rank, T] = sum_h cA_h @ Y^T_h ; out = r^T chunks.T @ cB^T,
           staged bf16 (host upcasts) to halve the output-DMA drain.

~283 us/core measured (baseline 607 us); rel err ~1.1e-2.
"""

import numpy as np
import ml_dtypes

import concourse.bass as bass
from concourse import bacc
import concourse.mybir as mybir
from concourse.tile import TileContext
from concourse.bass_utils import run_bass_kernel_spmd

B, T, C = 4, 2048, 1024
NH, HD = 16, 64
RANK = 128
NCORES = 8
HPC = 8          # heads per core
NPAIR = 4        # head pairs per core
G = 512          # C columns per head group
P = 128
F32 = mybir.dt.float32
BF16 = mybir.dt.bfloat16
I16 = mybir.dt.int16
BF = ml_dtypes.bfloat16

SCH_A = 23.083120654223414   # 128 * 0.125 * log2(e)
SCH_B = 16256.5              # 127*128 + 0.5 (trunc->round)
SCH_KB_MIN = 4
FILLER_N = 3
WARMUP_N = 150

_NC_CACHE = None


def _chunks(total, step):
    res = []
    o = 0
    while o < total:
        res.append((o, min(step, total - o)))
        o += min(step, total - o)
    return res


def build():
    nc = bacc.Bacc()
    dp = nc.declare_dram_parameter
    xT = dp("xT", [C, T], BF16, isOutput=False)
    qAT = dp("qAT", [C, RANK], BF16, isOutput=False)
    kAT = dp("kAT", [C, RANK], BF16, isOutput=False)
    vAT = dp("vAT", [C, RANK], BF16, isOutput=False)
    qBT = dp("qBT", [RANK, G], BF16, isOutput=False)
    kBT = dp("kBT", [RANK, G], BF16, isOutput=False)
    vBT = dp("vBT", [RANK, G], BF16, isOutput=False)
    cAT = dp("cAT", [G, RANK], BF16, isOutput=False)
    cBT = dp("cBT", [RANK, C], BF16, isOutput=False)
    mask01p = dp("mask01", [P, P], BF16, isOutput=False)
    out = dp("out", [T, C], BF16, isOutput=True)

    Exp = mybir.ActivationFunctionType.Exp
    MUL = mybir.AluOpType.mult
    ADD = mybir.AluOpType.add

    class _ScalarCopy:
        @staticmethod
        def tensor_copy(out, in_):
            return nc.scalar.copy(out, in_)

    def copy_eng(i):
        return _ScalarCopy if (i % 2 == 0) else nc.vector

    with TileContext(nc) as tc:
        with tc.tile_pool(name="sb", bufs=1) as sb:
            warm_src = sb.tile([P, 64], BF16, tag="warm")
            nc.gpsimd.memset(warm_src, 1.0)
            xT_cc = [sb.tile([P, T], BF16, tag=f"xT{cc}", name=f"xT{cc}")
                     for cc in range(8)]
            for cc in range(8):
                deng = nc.gpsimd if cc % 2 == 0 else nc.scalar
                deng.dma_start(
                    out=xT_cc[cc],
                    in_=xT.rearrange("(co ci) t -> ci co t", ci=P)[:, cc, :],
                )
            qAT_sb = sb.tile([P, 8, RANK], BF16, tag="qAT")
            nc.sync.dma_start(out=qAT_sb, in_=qAT.rearrange("(co ci) r -> ci co r", ci=P))
            kAT_sb = sb.tile([P, 8, RANK], BF16, tag="kAT")
            nc.sync.dma_start(out=kAT_sb, in_=kAT.rearrange("(co ci) r -> ci co r", ci=P))
            vAT_sb = sb.tile([P, 8, RANK], BF16, tag="vAT")
            nc.sync.dma_start(out=vAT_sb, in_=vAT.rearrange("(co ci) r -> ci co r", ci=P))
            qBT_sb = sb.tile([RANK, G], BF16, tag="qBT")
            nc.sync.dma_start(out=qBT_sb, in_=qBT[:, :])
            kBT_sb = sb.tile([RANK, G], BF16, tag="kBT")
            nc.sync.dma_start(out=kBT_sb, in_=kBT[:, :])
            vBT_sb = sb.tile([RANK, G], BF16, tag="vBT")
            nc.sync.dma_start(out=vBT_sb, in_=vBT[:, :])
            cAT_sb = sb.tile([64, HPC, RANK], BF16, tag="cAT")
            nc.sync.dma_start(out=cAT_sb, in_=cAT.rearrange("(h d) r -> d h r", d=64))
            cBT_sb = sb.tile([RANK, C], BF16, tag="cBT")
            nc.sync.dma_start(out=cBT_sb, in_=cBT[:, :])
            mask01 = sb.tile([P, P], BF16, tag="mask01")
            nc.sync.dma_start(out=mask01, in_=mask01p[:, :])

            QTp = [sb.tile([P, T], BF16, tag=f"QTp{p}", name=f"QTp{p}") for p in range(NPAIR)]
            KTp = [sb.tile([P, T], BF16, tag=f"KTp{p}", name=f"KTp{p}") for p in range(NPAIR)]
            YT = [sb.tile([64, T], BF16, tag=f"YT{h}", name=f"YT{h}") for h in range(HPC)]
            V_sb = sb.tile([P, 16, HPC, 65], BF16, tag="Vsb")
            tTq = sb.tile([P, T], BF16, tag="tTq")
            tTk = sb.tile([P, T], BF16, tag="tTk")
            tTv = sb.tile([P, T], BF16, tag="tTv")
            rT_sb = sb.tile([P, T], BF16, tag="rT")

            nc.gpsimd.memset(V_sb[:, :, :, 64:65], 1.0)

            # ---- PE warm-up during input DMA wait: ~6us of tiny matmuls ----
            with tc.tile_pool(name="wfil", bufs=1, space="PSUM") as wfil:
                wps = wfil.tile([4, 64], F32, tag="wps")
                for _w in range(WARMUP_N):
                    nc.tensor.matmul(wps, warm_src[:, 0:4], warm_src[:, 0:64],
                                     start=True, stop=True)

            # ---- phase A: t^T = A @ x^T for q,k,v ----
            with tc.tile_pool(name="psA", bufs=2, space="PSUM") as psA:
                for pi, (AT_sb, tT) in enumerate(((qAT_sb, tTq), (kAT_sb, tTk), (vAT_sb, tTv))):
                    pt = psA.tile([P, T], F32, tag="psA")
                    for cc in range(8):
                        for t0, tw in _chunks(T, 512):
                            nc.tensor.matmul(
                                pt[:, t0:t0 + tw],
                                AT_sb[:, cc, :],
                                xT_cc[cc][:, t0:t0 + tw],
                                start=(cc == 0), stop=(cc == 7),
                            )
                    copy_eng(pi).tensor_copy(out=tT, in_=pt)

            # ---- phase B: Q/K pair tiles (interleaved by pair), then V ----
            with (
                tc.tile_pool(name="psB", bufs=3, space="PSUM") as psB,
                tc.tile_pool(name="psV", bufs=3, space="PSUM") as psV,
            ):
                ci = 0
                for p in range(NPAIR):
                    for BT_sb, dest, tsrc in ((qBT_sb, QTp, tTq), (kBT_sb, KTp, tTk)):
                        for t0, tw in _chunks(T, 512):
                            p2 = psB.tile([P, 512], F32, tag="psB")
                            nc.tensor.matmul(
                                p2[:, :tw],
                                BT_sb[:, p * 128:(p + 1) * 128],
                                tsrc[:, t0:t0 + tw],
                                start=True, stop=True,
                            )
                            copy_eng(ci).tensor_copy(out=dest[p][:, t0:t0 + tw], in_=p2[:, :tw])
                            ci += 1

                # V keys-major with ones column
                for ti in range(16):
                    pv = psV.tile([P, G], F32, tag="psV")
                    nc.tensor.matmul(
                        pv, tTv[:, ti * 128:(ti + 1) * 128], vBT_sb,
                        start=True, stop=True,
                    )
                    copy_eng(ti).tensor_copy(
                        out=V_sb[:, ti, :, 0:64],
                        in_=pv.rearrange("p (h d) -> p h d", d=64),
                    )

            # ---- attention (j-major: one 512-query output bank at a time) ----
            with (
                tc.tile_pool(name="psS", bufs=2, space="PSUM") as psS,
                tc.tile_pool(name="psPV", bufs=3, space="PSUM") as psPV,
                tc.tile_pool(name="fil", bufs=1, space="PSUM") as filp,
                tc.tile_pool(name="ptp", bufs=4) as ptp,
                tc.tile_pool(name="nrm", bufs=4) as nrm,
                tc.tile_pool(name="bcp", bufs=4) as bcp,
            ):
                fps = filp.tile([4, 64], F32, tag="fil")
                for p in range(NPAIR):
                    for hc in range(2):
                        for j in (2 * hc, 2 * hc + 1):
                            r0 = 512 * j
                            pvt = {hh: psPV.tile([65, 512], F32, tag="pv",
                                                 name=f"pv{p}_{hh}_{j}")
                                   for hh in range(2)}
                            for kb in range(4 * j + 4):
                                qlo = max(1024 * hc, 128 * kb)
                                c0 = max(qlo, r0)
                                cw = r0 + 512 - c0
                                diag = (kb // 8 == hc)
                                use_sch = (not diag) and (kb >= SCH_KB_MIN)
                                has_diag_blk = diag and (kb // 4 == j)
                                ptile = ptp.tile([P, 2, 512], BF16, tag="pt")
                                s2 = psS.tile([P, 2, 512], F32, tag="s")
                                for hh in range(2):
                                    nc.tensor.matmul(
                                        s2[:, hh, 0:cw],
                                        KTp[p][64 * hh:64 * hh + 64, kb * 128:(kb + 1) * 128],
                                        QTp[p][64 * hh:64 * hh + 64, c0:c0 + cw],
                                        start=True, stop=True,
                                    )
                                if use_sch:
                                    nc.vector.tensor_scalar(
                                        out=ptile[:, :, 0:cw].bitcast(I16),
                                        in0=s2[:, :, 0:cw],
                                        scalar1=SCH_A, scalar2=SCH_B,
                                        op0=MUL, op1=ADD,
                                    )
                                else:
                                    nc.scalar.activation(
                                        ptile[:, :, 0:cw], s2[:, :, 0:cw],
                                        Exp, scale=0.125,
                                    )
                                if has_diag_blk:
                                    # zero the upper triangle of the diagonal block
                                    nc.vector.tensor_tensor(
                                        out=ptile[:, :, 0:P], in0=ptile[:, :, 0:P],
                                        in1=mask01.rearrange("k (one q) -> k one q", one=1).broadcast_to([P, 2, P]),
                                        op=MUL,
                                    )
                                for hh in range(2):
                                    h = 2 * p + hh
                                    nc.tensor.matmul(
                                        pvt[hh][:, c0 - r0:c0 - r0 + cw],
                                        V_sb[:, kb, h, :],
                                        ptile[:, hh, 0:cw],
                                        start=(kb == 0), stop=(kb == 4 * j + 3),
                                    )
                                nfil = FILLER_N if not use_sch else 1
                                for _f in range(nfil):
                                    nc.tensor.matmul(
                                        fps, warm_src[:, 0:4], warm_src[:, 0:64],
                                        start=True, stop=True, skip_group_check=True,
                                    )
                            for hh in range(2):
                                h = 2 * p + hh
                                den = nrm.tile([1, 512], F32, tag="den")
                                nc.vector.tensor_copy(
                                    out=den, in_=pvt[hh][64:65, :])
                                rec = nrm.tile([1, 512], F32, tag="rec")
                                nc.vector.reciprocal_approx_fast(
                                    out=rec, in_=den)
                                bc = bcp.tile([64, 512], F32, tag="bc")
                                nc.gpsimd.partition_broadcast(bc, rec)
                                nc.vector.tensor_mul(
                                    out=YT[h][:, r0:r0 + 512],
                                    in0=pvt[hh][0:64, :], in1=bc,
                                )

            # ---- phase D: c_proj ----
            with (
                tc.tile_pool(name="psD", bufs=1, space="PSUM") as psD,
                tc.tile_pool(name="psO", bufs=3, space="PSUM") as psO,
                tc.tile_pool(name="ost", bufs=3) as ost,
            ):
                pr = psD.tile([P, T], F32, tag="r")
                for h in range(HPC):
                    for t0, tw in _chunks(T, 512):
                        nc.tensor.matmul(
                            pr[:, t0:t0 + tw], cAT_sb[:, h, :], YT[h][:, t0:t0 + tw],
                            start=(h == 0), stop=(h == HPC - 1),
                        )
                for rci, (t0, tw) in enumerate(_chunks(T, 512)):
                    copy_eng(rci).tensor_copy(out=rT_sb[:, t0:t0 + tw], in_=pr[:, t0:t0 + tw])
                for tb in range(8):
                    ob = ost.tile([P, 2, 1024], BF16, tag="ob")
                    for sub in range(2):
                        ti = tb * 2 + sub
                        for nn in range(2):
                            po = psO.tile([P, 512], F32, tag="o")
                            nc.tensor.matmul(
                                po, rT_sb[:, ti * 128:(ti + 1) * 128],
                                cBT_sb[:, nn * 512:(nn + 1) * 512],
                                start=True, stop=True,
                            )
                            copy_eng(ti * 2 + nn).tensor_copy(
                                out=ob[:, sub, nn * 512:(nn + 1) * 512], in_=po)
                    deng = nc.sync if tb % 2 == 0 else nc.gpsimd
                    deng.dma_start(
                        out=out[tb * 256:(tb + 1) * 256, :].rearrange(
                            "(two p) c -> p two c", two=2),
                        in_=ob,
                    )
    nc.finalize()
    return nc


def make_in_maps(x, qA, qB, kA, kB, vA, vB, cA, cB):
    x, qA, qB, kA, kB, vA, vB, cA, cB = [
        np.asarray(a, dtype=np.float32) for a in (x, qA, qB, kA, kB, vA, vB, cA, cB)
    ]
    # mask01[k, q] = 1 if key k <= query q (valid causal), else 0
    mask01 = (np.arange(P)[:, None] <= np.arange(P)[None, :]).astype(BF)
    qATn = np.ascontiguousarray(qA.T).astype(BF)
    kATn = np.ascontiguousarray(kA.T).astype(BF)
    vATn = np.ascontiguousarray(vA.T).astype(BF)
    cBTn = np.ascontiguousarray(cB.T).astype(BF)
    in_maps = []
    for c in range(NCORES):
        b, g = divmod(c, 2)
        sl = slice(g * G, (g + 1) * G)
        in_maps.append({
            "xT": np.ascontiguousarray(x[b].T).astype(BF),
            "qAT": qATn, "kAT": kATn, "vAT": vATn,
            "qBT": np.ascontiguousarray(qB[sl, :].T).astype(BF),
            "kBT": np.ascontiguousarray(kB[sl, :].T).astype(BF),
            "vBT": np.ascontiguousarray(vB[sl, :].T).astype(BF),
            "cAT": np.ascontiguousarray(cA[:, sl].T).astype(BF),
            "cBT": cBTn,
            "mask01": mask01,
        })
    return in_maps


def combine(parts):
    return np.stack(
        [parts[2 * b].astype(np.float32) + parts[2 * b + 1].astype(np.float32)
         for b in range(B)], axis=0)


def kernel(x, qA, qB, kA, kB, vA, vB, cA, cB):
    global _NC_CACHE
    if _NC_CACHE is None:
        _NC_CACHE = build()
    in_maps = make_in_maps(x, qA, qB, kA, kB, vA, vB, cA, cB)
    res = run_bass_kernel_spmd(_NC_CACHE, in_maps, list(range(NCORES))).results
    return combine([res[c]["out"] for c in range(NCORES)])
